# revision 8
# baseline (speedup 1.0000x reference)
"""AdaConv Trainium2 kernel.

Computes, for x [B=32, C=256, H=64, W=64] and latent [B, C, 1, 1]:
    hw     = relu(latent @ w1.T + b1)
    scale  = hw @ w2.T + b2                    # [B, C]
    hb     = relu(latent @ bw1.T + bb1)
    bias   = hb @ bw2.T + bb2                  # [B, C]
    out    = x * scale[..., None, None] + bias[..., None, None]

Strategy: data-parallel over batch across 8 NeuronCores (4 samples each).
The small hypernetwork tensors (4x 256x256 weights, biases, latent) are
pre-laid-out host-side into a single [128, 2064] fp32 pack (weights
pre-transposed so the contraction dim lands on partitions) and loaded with
one DMA. The two tiny MLPs run on the TensorEngine (fp32), producing
scale/bias with (b,c) on partitions. The 16 MiB x shard then streams
through one fused VectorE tensor_scalar (x*scale + bias) per [128, 4096]
tile. x-in DMAs issue on SP, x-out DMAs on ACT so neither stream blocks
the other. Memory-bound: ~33.5 MB HBM traffic per core.
"""

from contextlib import ExitStack

import numpy as np

import concourse.bass as bass
import concourse.tile as tile
from concourse import bacc, mybir
from concourse.bass_utils import run_bass_kernel_spmd

B, C, H, W = 32, 256, 64, 64
N_CORES = 8
BL = B // N_CORES            # 4 samples per core
HWF = H * W                  # 4096
ROWS = BL * C                # 1024 (b, c) rows per core
P = 128
NCH = C // P                 # 2 chunks of 128 channels
N_ROW_TILES = ROWS // P      # 8 tiles of [128, 4096]
F32 = mybir.dt.float32

# wpack column layout: 4 transposed weights, then bias columns, then latent^T
W_OFF = {"w1": 0, "w2": 512, "bw1": 1024, "bw2": 1536}
B_OFF = {"b1": 2048, "b2": 2050, "bb1": 2052, "bb2": 2054}
L_OFF = 2056
PACK_COLS = L_OFF + NCH * BL  # 2064

_COMPILED_NC = None


def _mlp_branch(tc, pool, psum, wp, wkey1, bkey1, wkey2, bkey2, name):
    """Two-layer MLP on the packed transposed latent. Returns outT[oj] tiles
    [128, BL]: outT[oj][p, b] = (relu(l @ W1.T + b1) @ W2.T + b2)[b, oj*128+p]."""
    nc = tc.nc
    o1, o2 = W_OFF[wkey1], W_OFF[wkey2]
    h1T = []
    for hj in range(NCH):
        ps = psum.tile([P, BL], F32, tag="ps_mm")
        for ci in range(NCH):
            nc.tensor.matmul(
                ps[:],
                wp[:, o1 + ci * C + hj * P: o1 + ci * C + (hj + 1) * P],
                wp[:, L_OFF + ci * BL: L_OFF + (ci + 1) * BL],
                start=(ci == 0), stop=(ci == NCH - 1),
            )
        h = pool.tile([P, BL], F32, tag=f"{name}_h{hj}")
        # h = max(ps + b1_col, 0)  (fused relu on DVE)
        nc.vector.tensor_scalar(
            h[:], ps[:], wp[:, B_OFF[bkey1] + hj: B_OFF[bkey1] + hj + 1], 0.0,
            mybir.AluOpType.add, mybir.AluOpType.max,
        )
        h1T.append(h)
    outT = []
    for oj in range(NCH):
        ps = psum.tile([P, BL], F32, tag="ps_mm")
        for hi in range(NCH):
            nc.tensor.matmul(
                ps[:],
                wp[:, o2 + hi * C + oj * P: o2 + hi * C + (oj + 1) * P],
                h1T[hi][:],
                start=(hi == 0), stop=(hi == NCH - 1),
            )
        o = pool.tile([P, BL], F32, tag=f"{name}_o{oj}")
        nc.vector.tensor_scalar(
            o[:], ps[:], wp[:, B_OFF[bkey2] + oj: B_OFF[bkey2] + oj + 1], None,
            mybir.AluOpType.add,
        )
        outT.append(o)
    return outT


def _build_body(ctx, tc, aps):
    nc = tc.nc
    x, out = aps["x"], aps["out"]

    const = ctx.enter_context(tc.tile_pool(name="const", bufs=1))
    mlp_pool = ctx.enter_context(tc.tile_pool(name="mlp", bufs=1))
    psum = ctx.enter_context(tc.tile_pool(name="psum", bufs=2, space="PSUM"))

    wp = const.tile([P, PACK_COLS], F32)
    # ACT queue: keeps SP free so the first x-in DMA is SP's first issue
    nc.sync.dma_start(wp[:], aps["wpack"][:, :])

    scaleT = _mlp_branch(tc, mlp_pool, psum, wp, "w1", "b1", "w2", "b2", "sc")
    biasT = _mlp_branch(tc, mlp_pool, psum, wp, "bw1", "bb1", "bw2", "bb2", "bi")

    # stream x: row r = b*C + c ; tile t covers rows [t*128, (t+1)*128)
    xpool = ctx.enter_context(tc.tile_pool(name="x", bufs=6))
    for t in range(N_ROW_TILES):
        b, half = divmod(t, NCH)
        xt = xpool.tile([P, HWF], F32)
        in_eng = nc.sync if t % 2 == 0 else nc.scalar
        out_eng = nc.scalar if t % 2 == 0 else nc.sync
        in_eng.dma_start(xt[:], x[t * P:(t + 1) * P, :])
        nc.vector.tensor_scalar(
            xt[:], xt[:],
            scaleT[half][:, b:b + 1], biasT[half][:, b:b + 1],
            mybir.AluOpType.mult, mybir.AluOpType.add,
        )
        out_eng.dma_start(out[t * P:(t + 1) * P, :], xt[:])


def build_nc():
    nc = bacc.Bacc("TRN2", debug=False, num_devices=N_CORES)
    aps = {
        "x": nc.declare_dram_parameter("x", [ROWS, HWF], F32, isOutput=False).ap(),
        "wpack": nc.declare_dram_parameter(
            "wpack", [P, PACK_COLS], F32, isOutput=False
        ).ap(),
        "out": nc.declare_dram_parameter("out", [ROWS, HWF], F32, isOutput=True).ap(),
    }
    with tile.TileContext(nc) as tc, ExitStack() as ctx:
        _build_body(ctx, tc, aps)
    nc.compile()
    return nc


def _get_nc():
    global _COMPILED_NC
    if _COMPILED_NC is None:
        _COMPILED_NC = build_nc()
    return _COMPILED_NC


def _make_wpack(inputs, core):
    """[128, PACK_COLS] fp32: transposed weights, bias columns, latent^T."""
    wp = np.empty((P, PACK_COLS), dtype=np.float32)
    for k in ("w1", "w2", "bw1", "bw2"):
        wT = np.asarray(inputs[k], dtype=np.float32).T  # [in(c), out]
        o = W_OFF[k]
        for ci in range(NCH):
            wp[:, o + ci * C: o + (ci + 1) * C] = wT[ci * P:(ci + 1) * P, :]
    for k in ("b1", "b2", "bb1", "bb2"):
        bcol = np.asarray(inputs[k], dtype=np.float32).reshape(NCH, P).T  # [128, 2]
        wp[:, B_OFF[k]: B_OFF[k] + NCH] = bcol
    lat = np.asarray(inputs["latent"], dtype=np.float32).reshape(B, C)
    lT = lat[core * BL:(core + 1) * BL, :].T  # [C, BL]
    for ci in range(NCH):
        wp[:, L_OFF + ci * BL: L_OFF + (ci + 1) * BL] = lT[ci * P:(ci + 1) * P, :]
    return wp


def make_in_maps(inputs):
    x = np.ascontiguousarray(np.asarray(inputs["x"], dtype=np.float32))
    in_maps = []
    for i in range(N_CORES):
        in_maps.append({
            "x": np.ascontiguousarray(x[i * BL:(i + 1) * BL]).reshape(ROWS, HWF),
            "wpack": _make_wpack(inputs, i),
        })
    return in_maps


def run(inputs, trace=False, **kwargs):
    """Run on 8 NeuronCores. Returns (full_output, BassKernelResults)."""
    nc = _get_nc()
    in_maps = make_in_maps(inputs)
    res = run_bass_kernel_spmd(
        nc, in_maps, core_ids=list(range(N_CORES)), trace=trace, **kwargs
    )
    shards = [
        np.asarray(res.results[i]["out"], dtype=np.float32).reshape(BL, C, H, W)
        for i in range(N_CORES)
    ]
    return np.concatenate(shards, axis=0), res


def kernel(**inputs):
    out, _ = run(inputs, trace=False)
    return out


# revision 14
# speedup vs baseline: 1.0295x; 1.0295x over previous
"""AdaConv Trainium2 kernel.

Computes, for x [B=32, C=256, H=64, W=64] and latent [B, C, 1, 1]:
    hw     = relu(latent @ w1.T + b1)
    scale  = hw @ w2.T + b2                    # [B, C]
    hb     = relu(latent @ bw1.T + bb1)
    bias   = hb @ bw2.T + bb2                  # [B, C]
    out    = x * scale[..., None, None] + bias[..., None, None]

Strategy: data-parallel over batch across 8 NeuronCores (4 samples each).
The small hypernetwork tensors (4x 256x256 weights, biases, latent) are
pre-laid-out host-side into a single [128, 2064] fp32 pack (weights
pre-transposed so the contraction dim lands on partitions) and loaded with
one DMA. The two tiny MLPs run on the TensorEngine (fp32), producing
scale/bias with (b,c) on partitions. The 16 MiB x shard then streams
through one fused VectorE tensor_scalar (x*scale + bias) per [128, 4096]
tile. x-in DMAs issue on SP, x-out DMAs on ACT so neither stream blocks
the other. Memory-bound: ~33.5 MB HBM traffic per core.
"""

from contextlib import ExitStack

import numpy as np

import concourse.bass as bass
import concourse.tile as tile
from concourse import bacc, mybir
from concourse.bass_utils import run_bass_kernel_spmd

B, C, H, W = 32, 256, 64, 64
N_CORES = 8
BL = B // N_CORES            # 4 samples per core
HWF = H * W                  # 4096
ROWS = BL * C                # 1024 (b, c) rows per core
P = 128
NCH = C // P                 # 2 chunks of 128 channels
N_ROW_TILES = ROWS // P      # 8 tiles of [128, 4096]
F32 = mybir.dt.float32

# wpack column layout: 4 transposed weights, then bias columns, then latent^T
W_OFF = {"w1": 0, "w2": 512, "bw1": 1024, "bw2": 1536}
B_OFF = {"b1": 2048, "b2": 2050, "bb1": 2052, "bb2": 2054}
L_OFF = 2056
PACK_COLS = L_OFF + NCH * BL  # 2064

_COMPILED_NC = None


def _mlp_branch(tc, pool, psum, wp, wkey1, bkey1, wkey2, bkey2, name):
    """Two-layer MLP on the packed transposed latent. Returns outT[oj] tiles
    [128, BL]: outT[oj][p, b] = (relu(l @ W1.T + b1) @ W2.T + b2)[b, oj*128+p]."""
    nc = tc.nc
    o1, o2 = W_OFF[wkey1], W_OFF[wkey2]
    h1T = []
    for hj in range(NCH):
        ps = psum.tile([P, BL], F32, tag="ps_mm")
        for ci in range(NCH):
            nc.tensor.matmul(
                ps[:],
                wp[:, o1 + ci * C + hj * P: o1 + ci * C + (hj + 1) * P],
                wp[:, L_OFF + ci * BL: L_OFF + (ci + 1) * BL],
                start=(ci == 0), stop=(ci == NCH - 1),
            )
        h = pool.tile([P, BL], F32, tag=f"{name}_h{hj}")
        # h = max(ps + b1_col, 0)  (fused relu on DVE)
        nc.vector.tensor_scalar(
            h[:], ps[:], wp[:, B_OFF[bkey1] + hj: B_OFF[bkey1] + hj + 1], 0.0,
            mybir.AluOpType.add, mybir.AluOpType.max,
        )
        h1T.append(h)
    outT = []
    for oj in range(NCH):
        ps = psum.tile([P, BL], F32, tag="ps_mm")
        for hi in range(NCH):
            nc.tensor.matmul(
                ps[:],
                wp[:, o2 + hi * C + oj * P: o2 + hi * C + (oj + 1) * P],
                h1T[hi][:],
                start=(hi == 0), stop=(hi == NCH - 1),
            )
        o = pool.tile([P, BL], F32, tag=f"{name}_o{oj}")
        nc.vector.tensor_scalar(
            o[:], ps[:], wp[:, B_OFF[bkey2] + oj: B_OFF[bkey2] + oj + 1], None,
            mybir.AluOpType.add,
        )
        outT.append(o)
    return outT


def _build_body(ctx, tc, aps):
    nc = tc.nc
    x, out = aps["x"], aps["out"]

    const = ctx.enter_context(tc.tile_pool(name="const", bufs=1))
    mlp_pool = ctx.enter_context(tc.tile_pool(name="mlp", bufs=1))
    psum = ctx.enter_context(tc.tile_pool(name="psum", bufs=2, space="PSUM"))

    wp = const.tile([P, PACK_COLS], F32)
    nc.sync.dma_start(wp[:], aps["wpack"][:, :])

    scaleT = _mlp_branch(tc, mlp_pool, psum, wp, "w1", "b1", "w2", "b2", "sc")
    biasT = _mlp_branch(tc, mlp_pool, psum, wp, "bw1", "bb1", "bw2", "bb2", "bi")

    # stream x: row r = b*C + c ; tile t covers rows [t*128, (t+1)*128)
    xpool = ctx.enter_context(tc.tile_pool(name="x", bufs=8))
    for t in range(N_ROW_TILES):
        b, half = divmod(t, NCH)
        xt = xpool.tile([P, HWF], F32)
        nc.sync.dma_start(xt[:], x[t * P:(t + 1) * P, :])
        nc.vector.tensor_scalar(
            xt[:], xt[:],
            scaleT[half][:, b:b + 1], biasT[half][:, b:b + 1],
            mybir.AluOpType.mult, mybir.AluOpType.add,
        )
        nc.scalar.dma_start(out[t * P:(t + 1) * P, :], xt[:])


def build_nc():
    nc = bacc.Bacc("TRN2", debug=False, num_devices=N_CORES)
    aps = {
        "x": nc.declare_dram_parameter("x", [ROWS, HWF], F32, isOutput=False).ap(),
        "wpack": nc.declare_dram_parameter(
            "wpack", [P, PACK_COLS], F32, isOutput=False
        ).ap(),
        "out": nc.declare_dram_parameter("out", [ROWS, HWF], F32, isOutput=True).ap(),
    }
    with tile.TileContext(nc) as tc, ExitStack() as ctx:
        _build_body(ctx, tc, aps)
    nc.compile()
    return nc


def _get_nc():
    global _COMPILED_NC
    if _COMPILED_NC is None:
        _COMPILED_NC = build_nc()
    return _COMPILED_NC


def _make_wpack(inputs, core):
    """[128, PACK_COLS] fp32: transposed weights, bias columns, latent^T."""
    wp = np.empty((P, PACK_COLS), dtype=np.float32)
    for k in ("w1", "w2", "bw1", "bw2"):
        wT = np.asarray(inputs[k], dtype=np.float32).T  # [in(c), out]
        o = W_OFF[k]
        for ci in range(NCH):
            wp[:, o + ci * C: o + (ci + 1) * C] = wT[ci * P:(ci + 1) * P, :]
    for k in ("b1", "b2", "bb1", "bb2"):
        bcol = np.asarray(inputs[k], dtype=np.float32).reshape(NCH, P).T  # [128, 2]
        wp[:, B_OFF[k]: B_OFF[k] + NCH] = bcol
    lat = np.asarray(inputs["latent"], dtype=np.float32).reshape(B, C)
    lT = lat[core * BL:(core + 1) * BL, :].T  # [C, BL]
    for ci in range(NCH):
        wp[:, L_OFF + ci * BL: L_OFF + (ci + 1) * BL] = lT[ci * P:(ci + 1) * P, :]
    return wp


def make_in_maps(inputs):
    x = np.ascontiguousarray(np.asarray(inputs["x"], dtype=np.float32))
    in_maps = []
    for i in range(N_CORES):
        in_maps.append({
            "x": np.ascontiguousarray(x[i * BL:(i + 1) * BL]).reshape(ROWS, HWF),
            "wpack": _make_wpack(inputs, i),
        })
    return in_maps


def run(inputs, trace=False, **kwargs):
    """Run on 8 NeuronCores. Returns (full_output, BassKernelResults)."""
    nc = _get_nc()
    in_maps = make_in_maps(inputs)
    res = run_bass_kernel_spmd(
        nc, in_maps, core_ids=list(range(N_CORES)), trace=trace, **kwargs
    )
    shards = [
        np.asarray(res.results[i]["out"], dtype=np.float32).reshape(BL, C, H, W)
        for i in range(N_CORES)
    ]
    return np.concatenate(shards, axis=0), res


def kernel(**inputs):
    out, _ = run(inputs, trace=False)
    return out


# revision 16
# speedup vs baseline: 1.0821x; 1.0511x over previous
"""AdaConv Trainium2 kernel.

Computes, for x [B=32, C=256, H=64, W=64] and latent [B, C, 1, 1]:
    hw     = relu(latent @ w1.T + b1)
    scale  = hw @ w2.T + b2                    # [B, C]
    hb     = relu(latent @ bw1.T + bb1)
    bias   = hb @ bw2.T + bb2                  # [B, C]
    out    = x * scale[..., None, None] + bias[..., None, None]

Strategy: data-parallel over batch across 8 NeuronCores (4 samples each).
The small hypernetwork tensors (4x 256x256 weights, biases, latent) are
pre-laid-out host-side into a single [128, 2064] fp32 pack (weights
pre-transposed so the contraction dim lands on partitions) and loaded with
one DMA. The two tiny MLPs run on the TensorEngine (fp32), producing
scale/bias with (b,c) on partitions. The 16 MiB x shard then streams
through one fused VectorE tensor_scalar (x*scale + bias) per [128, 4096]
tile. x-in DMAs issue on SP, x-out DMAs on ACT so neither stream blocks
the other. Memory-bound: ~33.5 MB HBM traffic per core.
"""

from contextlib import ExitStack

import numpy as np

import concourse.bass as bass
import concourse.tile as tile
from concourse import bacc, mybir
from concourse.bass_utils import run_bass_kernel_spmd

B, C, H, W = 32, 256, 64, 64
N_CORES = 8
BL = B // N_CORES            # 4 samples per core
HWF = H * W                  # 4096
ROWS = BL * C                # 1024 (b, c) rows per core
P = 128
NCH = C // P                 # 2 chunks of 128 channels
N_ROW_TILES = ROWS // P      # 8 tiles of [128, 4096]
F32 = mybir.dt.float32

# wpack column layout: 4 transposed weights, then bias columns, then latent^T
W_OFF = {"w1": 0, "w2": 512, "bw1": 1024, "bw2": 1536}
B_OFF = {"b1": 2048, "b2": 2050, "bb1": 2052, "bb2": 2054}
L_OFF = 2056
PACK_COLS = L_OFF + NCH * BL  # 2064

_COMPILED_NC = None


def _mlp_branch(tc, pool, psum, wp, wkey1, bkey1, wkey2, bkey2, name):
    """Two-layer MLP on the packed transposed latent. Returns outT[oj] tiles
    [128, BL]: outT[oj][p, b] = (relu(l @ W1.T + b1) @ W2.T + b2)[b, oj*128+p]."""
    nc = tc.nc
    o1, o2 = W_OFF[wkey1], W_OFF[wkey2]
    h1T = []
    for hj in range(NCH):
        ps = psum.tile([P, BL], F32, tag="ps_mm")
        for ci in range(NCH):
            nc.tensor.matmul(
                ps[:],
                wp[:, o1 + ci * C + hj * P: o1 + ci * C + (hj + 1) * P],
                wp[:, L_OFF + ci * BL: L_OFF + (ci + 1) * BL],
                start=(ci == 0), stop=(ci == NCH - 1),
            )
        h = pool.tile([P, BL], F32, tag=f"{name}_h{hj}")
        # h = max(ps + b1_col, 0)  (fused relu on DVE)
        nc.vector.tensor_scalar(
            h[:], ps[:], wp[:, B_OFF[bkey1] + hj: B_OFF[bkey1] + hj + 1], 0.0,
            mybir.AluOpType.add, mybir.AluOpType.max,
        )
        h1T.append(h)
    outT = []
    for oj in range(NCH):
        ps = psum.tile([P, BL], F32, tag="ps_mm")
        for hi in range(NCH):
            nc.tensor.matmul(
                ps[:],
                wp[:, o2 + hi * C + oj * P: o2 + hi * C + (oj + 1) * P],
                h1T[hi][:],
                start=(hi == 0), stop=(hi == NCH - 1),
            )
        o = pool.tile([P, BL], F32, tag=f"{name}_o{oj}")
        nc.vector.tensor_scalar(
            o[:], ps[:], wp[:, B_OFF[bkey2] + oj: B_OFF[bkey2] + oj + 1], None,
            mybir.AluOpType.add,
        )
        outT.append(o)
    return outT


def _build_body(ctx, tc, aps):
    nc = tc.nc
    x, out = aps["x"], aps["out"]

    const = ctx.enter_context(tc.tile_pool(name="const", bufs=1))
    mlp_pool = ctx.enter_context(tc.tile_pool(name="mlp", bufs=1))
    psum = ctx.enter_context(tc.tile_pool(name="psum", bufs=2, space="PSUM"))

    wp = const.tile([P, PACK_COLS], F32)
    nc.sync.dma_start(wp[:], aps["wpack"][:, :])

    scaleT = _mlp_branch(tc, mlp_pool, psum, wp, "w1", "b1", "w2", "b2", "sc")
    biasT = _mlp_branch(tc, mlp_pool, psum, wp, "bw1", "bb1", "bw2", "bb2", "bi")

    # stream x: row r = b*C + c ; tile t covers rows [t*128, (t+1)*128)
    xpool = ctx.enter_context(tc.tile_pool(name="x", bufs=8))
    for t in range(N_ROW_TILES):
        b, half = divmod(t, NCH)
        xt = xpool.tile([P, HWF], F32)
        nc.sync.dma_start(xt[:], x[t * P:(t + 1) * P, :])
        nc.vector.tensor_scalar(
            xt[:], xt[:],
            scaleT[half][:, b:b + 1], biasT[half][:, b:b + 1],
            mybir.AluOpType.mult, mybir.AluOpType.add,
        )
        nc.scalar.dma_start(out[t * P:(t + 1) * P, :], xt[:])


def build_nc():
    nc = bacc.Bacc("TRN2", debug=False, num_devices=N_CORES)
    aps = {
        "x": nc.declare_dram_parameter("x", [ROWS, HWF], F32, isOutput=False).ap(),
        "wpack": nc.declare_dram_parameter(
            "wpack", [P, PACK_COLS], F32, isOutput=False
        ).ap(),
        "out": nc.declare_dram_parameter("out", [ROWS, HWF], F32, isOutput=True).ap(),
    }
    with tile.TileContext(nc) as tc, ExitStack() as ctx:
        _build_body(ctx, tc, aps)
    nc.compile()
    return nc


def _get_nc():
    global _COMPILED_NC
    if _COMPILED_NC is None:
        _COMPILED_NC = build_nc()
    return _COMPILED_NC


def _make_wpack(inputs, core):
    """[128, PACK_COLS] fp32: transposed weights, bias columns, latent^T."""
    wp = np.empty((P, PACK_COLS), dtype=np.float32)
    for k in ("w1", "w2", "bw1", "bw2"):
        wT = np.asarray(inputs[k], dtype=np.float32).T  # [in(c), out]
        o = W_OFF[k]
        for ci in range(NCH):
            wp[:, o + ci * C: o + (ci + 1) * C] = wT[ci * P:(ci + 1) * P, :]
    for k in ("b1", "b2", "bb1", "bb2"):
        bcol = np.asarray(inputs[k], dtype=np.float32).reshape(NCH, P).T  # [128, 2]
        wp[:, B_OFF[k]: B_OFF[k] + NCH] = bcol
    lat = np.asarray(inputs["latent"], dtype=np.float32).reshape(B, C)
    lT = lat[core * BL:(core + 1) * BL, :].T  # [C, BL]
    for ci in range(NCH):
        wp[:, L_OFF + ci * BL: L_OFF + (ci + 1) * BL] = lT[ci * P:(ci + 1) * P, :]
    return wp


def make_in_maps(inputs):
    x = np.ascontiguousarray(np.asarray(inputs["x"], dtype=np.float32))
    in_maps = []
    for i in range(N_CORES):
        in_maps.append({
            "x": np.ascontiguousarray(x[i * BL:(i + 1) * BL]).reshape(ROWS, HWF),
            "wpack": _make_wpack(inputs, i),
        })
    return in_maps


def run(inputs, trace=False, **kwargs):
    """Run on 8 NeuronCores. Returns (full_output, BassKernelResults)."""
    nc = _get_nc()
    in_maps = make_in_maps(inputs)
    res = run_bass_kernel_spmd(
        nc, in_maps, core_ids=list(range(N_CORES)), trace=trace, **kwargs
    )
    shards = [
        np.asarray(res.results[i]["out"], dtype=np.float32).reshape(BL, C, H, W)
        for i in range(N_CORES)
    ]
    return np.concatenate(shards, axis=0), res


def kernel(**inputs):
    out, _ = run(inputs, trace=False)
    return out
